# revision 4
# baseline (speedup 1.0000x reference)
"""Trainium2 Bass kernel for nn_CDER_64493228917301 (gnn_message_passing).

Reference semantics (GATConv-style, DGL u_dot_v / v_mul_e):
    el  = (e_ft @ W.T).reshape(N, H, F)
    e   = leaky_relu(einsum('ehf,ehf->eh', el[src], el[dst]))
    a   = segment_softmax(e, dst)          # softmax over edges sharing dst
    msg = ft[dst] * a[:, :, None]          # NOTE: uses DESTINATION features
    out = (segment_sum(msg, dst) + bias.reshape(1,H,F)).mean(axis=1)

Key algebraic identity: because the message uses ft[dst] (not ft[src]),
every edge in dst-segment n contributes ft[n] * a_e, and the softmax
weights a_e of one segment sum to 1.  Hence

    segment_sum(msg, dst)[n] = ft[n] * (1 if node n has >=1 in-edge else 0)

exactly (up to f32 rounding of order 1e-7 -- verified global rel err
1.2e-7 vs the jax reference).  The attention logits, the e_ft @ W matmul
and the edge gathers cancel out of the output entirely; the only thing
the edge list contributes is the per-node "has in-edge" indicator.

So the kernel computes, fully on device:

    out[n, f] = (sum_h ft[n, h, f]) * fscale[n] + bias_mean[f]

where fscale[n] = 0.25 * has_in_edge[n] (the 1/H fold is free) and
bias_mean = bias.reshape(H, F).mean(0).  The indicator is produced on
the host during input sharding (a single vectorized scatter over dst --
index preprocessing, like the sharding itself).

Distribution: node-parallel across the 8 NeuronCores.  Each core gets a
12500-node shard (padded to 12544 = 98*128) and streams its 6.4 MB of
ft through SBUF, which makes the kernel purely HBM-bandwidth-bound --
the target regime.

Implementation is raw Bass (no Tile framework, no Block) with manual
semaphores; the Tile scheduler's entry/exit drain + all-engine barriers
cost ~15 us on a ~25 us kernel.  Pipeline (NBUF-deep rotating buffers):
  - SP (sync) HWDGE ring:    7x 896 KB ft tile loads, free-running
  - ACT (scalar) HWDGE ring: fscale/bias const loads + 7x 229 KB stores
    (separate ring so stores' sem-waits never block load issue)
  - DVE (vector) per tile:   u=h0+h2, v=h1+h3, o=u+v, o*=fscale_x
  - GpSimd per tile:         fscale broadcast-expand (so DVE's multiply
    reads a contiguous operand) and the bias add; plus the end-of-kernel
    semaphore range clear (gated on per-engine done incs) so the NEFF
    stays re-executable.

Semaphore-counting note: one HWDGE DMA fans its 16 per-SDMA-engine
slices' completions into the sem as +1 each; "sem >= 16*m" therefore
proves ALL first m DMAs on that ring are complete (every engine must
have retired its slice of every one of them), but "sem >= 16" with two
DMAs in flight does NOT prove the first one finished.
"""

import numpy as np

N = 100000
H = 4
F = 32
D = H * F            # 128 floats per node in ft
NC = 8               # cores
PER = N // NC        # 12500 nodes per core
P = 128              # SBUF partitions
G = 14               # node-groups per partition per tile
B = 7                # tiles per core
PAD = P * G * B      # 12544 padded nodes per core
NBUF = 3             # ft / out buffer slots

_cached = None


def _build_bass():
    import concourse.bass as bass
    from concourse import mybir

    f32 = mybir.dt.float32
    nc = bass.Bass(
        "TRN2",
        target_bir_lowering=False,
        debug=False,
        num_devices=NC,
    )
    ft_in = nc.dram_tensor("ft_in", [PAD, D], f32, kind="ExternalInput").ap()
    fs_in = nc.dram_tensor("fs_in", [PAD], f32, kind="ExternalInput").ap()
    bias_in = nc.dram_tensor("bias_in", [P, F], f32, kind="ExternalInput").ap()
    out = nc.dram_tensor("out", [PAD, F], f32, kind="ExternalOutput").ap()

    # node index n (within the core's shard) = p*(G*B) + b*G + g
    ftv = ft_in.rearrange("(p b g) d -> b p (g d)", p=P, b=B, g=G)   # [B,128,G*D]
    fsv = fs_in.rearrange("(p x) -> p x", p=P)                        # [128, B*G]
    outv = out.rearrange("(p b g) f -> b p (g f)", p=P, b=B, g=G)     # [B,128,G*F]

    sem_ft = nc.alloc_semaphore("sem_ft")      # ft loads      (+16 each, SP ring)
    sem_cb = nc.alloc_semaphore("sem_cb")      # consts+stores (+16 each, ACT ring)
    sem_exp = nc.alloc_semaphore("sem_exp")    # gpsimd fscale-expand done (+1)
    sem_v4 = nc.alloc_semaphore("sem_v4")      # vector finished tile (+1)
    sem_comp = nc.alloc_semaphore("sem_comp")  # gpsimd bias-add done = tile done (+1)
    sem_done = nc.alloc_semaphore("sem_done")  # per-engine finished (+1)
    ALL_SEMS = [sem_ft, sem_cb, sem_exp, sem_v4, sem_comp, sem_done]

    GD, GF = G * D, G * F

    with (
        nc.sbuf_tensor("ft_buf", [P, NBUF * GD], f32) as ft_buf,
        nc.sbuf_tensor("u_buf", [P, 2 * GF], f32) as u_buf,
        nc.sbuf_tensor("o_buf", [P, NBUF * GF], f32) as o_buf,
        nc.sbuf_tensor("fs_buf", [P, B * G], f32) as fs_buf,
        nc.sbuf_tensor("bias_buf", [P, F], f32) as bias_buf,
        nc.sbuf_tensor("fsx_buf", [P, 2 * GF], f32) as fsx_buf,
        nc.sbuf_tensor("biasx_buf", [P, GF], f32) as biasx_buf,
    ):
        def o3(b):
            return (
                o_buf[:, (b % NBUF) * GF : (b % NBUF + 1) * GF]
                .rearrange("p (g f) -> p g f", f=F)
            )

        def fsx3(b):
            return (
                fsx_buf[:, (b % 2) * GF : (b % 2 + 1) * GF]
                .rearrange("p (g f) -> p g f", f=F)
            )

        # ---- SP ring: ft tile loads ------------------------------------
        for b in range(B):
            ld = nc.sync.dma_start(
                ft_buf[:, (b % NBUF) * GD : (b % NBUF + 1) * GD], ftv[b]
            )
            if b >= NBUF:
                # slot free once vector consumed tile b-NBUF
                ld._wait_ge(sem_v4, b - NBUF + 1)
            ld.then_inc(sem_ft, 16)
        nc.sync.wait_ge(sem_ft, 16 * B).then_inc(sem_done, 1)

        # ---- ACT ring: consts then stores ------------------------------
        nc.scalar.dma_start(fs_buf[:], fsv).then_inc(sem_cb, 16)
        nc.scalar.dma_start(bias_buf[:], bias_in).then_inc(sem_cb, 16)
        for b in range(B):
            st = nc.scalar.dma_start(
                outv[b], o_buf[:, (b % NBUF) * GF : (b % NBUF + 1) * GF]
            )
            st._wait_ge(sem_comp, b + 1)
            st.then_inc(sem_cb, 16)
        nc.scalar.wait_ge(sem_cb, 16 * (B + 2)).then_inc(sem_done, 1)

        # ---- DVE: head sums + fscale multiply --------------------------
        for b in range(B):
            ft_t = ft_buf[:, (b % NBUF) * GD : (b % NBUF + 1) * GD]
            fth = ft_t.rearrange("p (g hh f) -> p hh g f", g=G, hh=H)
            u2 = u_buf[:, :GF].rearrange("p (g f) -> p g f", f=F)
            v2 = u_buf[:, GF:].rearrange("p (g f) -> p g f", f=F)
            op1 = nc.vector.tensor_add(u2, fth[:, 0], fth[:, 2])
            op1._wait_ge(sem_ft, 16 * (b + 1))
            nc.vector.tensor_add(v2, fth[:, 1], fth[:, 3])
            op3 = nc.vector.tensor_add(o3(b), u2, v2)
            if b >= NBUF:
                # o slot free once store of tile b-NBUF retired
                # (store b-NBUF is DMA #(2 + b - NBUF + 1) on the ACT ring)
                op3._wait_ge(sem_cb, 16 * (2 + b - NBUF + 1))
            op4 = nc.vector.tensor_mul(o3(b), o3(b), fsx3(b))
            op4._wait_ge(sem_exp, b + 1)
            op4.then_inc(sem_v4, 1)
        nc.vector.wait_ge(sem_v4, B).then_inc(sem_done, 1)

        # ---- GpSimd: broadcast expands + bias adds + final sem clear ---
        bx = biasx_buf[:].rearrange("p (g f) -> p g f", f=F)
        cp = nc.gpsimd.tensor_copy(
            bx, bias_buf[:].unsqueeze(1).broadcast_to([P, G, F])
        )
        cp._wait_ge(sem_cb, 32)  # both const DMAs fully retired
        for b in range(B):
            fs_bc = (
                fs_buf[:, b * G : (b + 1) * G].unsqueeze(2).broadcast_to([P, G, F])
            )
            ex = nc.gpsimd.tensor_copy(fsx3(b), fs_bc)
            if b >= 2:
                # fsx slot free once vector's multiply of tile b-2 is done
                ex._wait_ge(sem_v4, b - 1)
            ex.then_inc(sem_exp, 1)
            ba = nc.gpsimd.tensor_add(o3(b), o3(b), bx)
            ba._wait_ge(sem_v4, b + 1)
            ba.then_inc(sem_comp, 1)
        # end-of-kernel: wait for every engine, then zero all semaphores so
        # the loaded NEFF can be executed again.
        nc.gpsimd.sem_clear(ALL_SEMS[0])._wait_ge(sem_done, 3)
        for s in ALL_SEMS[1:]:
            nc.gpsimd.sem_clear(s)

    return nc


# results of the last device run (for test harness introspection)
LAST_RESULTS = None


def kernel(ft, e_ft, W, bias, src, dst):
    global _cached, LAST_RESULTS
    from concourse import bass_utils

    ft = np.ascontiguousarray(np.asarray(ft, dtype=np.float32)).reshape(N, D)
    bias = np.asarray(bias, dtype=np.float32)
    dst = np.asarray(dst)

    # per-node in-edge indicator, folded with the 1/H of the head mean
    fscale = np.zeros(N, np.float32)
    fscale[dst] = 1.0 / H
    bias_mean = bias.reshape(H, F).mean(axis=0)
    bias_b = np.ascontiguousarray(np.broadcast_to(bias_mean, (P, F)))

    in_maps = []
    for c in range(NC):
        ft_s = np.zeros((PAD, D), np.float32)
        ft_s[:PER] = ft[c * PER : (c + 1) * PER]
        fs_s = np.zeros(PAD, np.float32)
        fs_s[:PER] = fscale[c * PER : (c + 1) * PER]
        in_maps.append({"ft_in": ft_s, "fs_in": fs_s, "bias_in": bias_b})

    if _cached is None:
        _cached = _build_bass()
    nc = _cached

    res = bass_utils.run_bass_kernel_spmd(nc, in_maps, core_ids=list(range(NC)))
    LAST_RESULTS = res
    out = np.empty((N, F), np.float32)
    for c in range(NC):
        out[c * PER : (c + 1) * PER] = res.results[c]["out"][:PER]
    return out


# revision 5
# speedup vs baseline: 1.2942x; 1.2942x over previous
"""Trainium2 Bass kernel for nn_CDER_64493228917301 (gnn_message_passing).

Reference semantics (GATConv-style, DGL u_dot_v / v_mul_e):
    el  = (e_ft @ W.T).reshape(N, H, F)
    e   = leaky_relu(einsum('ehf,ehf->eh', el[src], el[dst]))
    a   = segment_softmax(e, dst)          # softmax over edges sharing dst
    msg = ft[dst] * a[:, :, None]          # NOTE: uses DESTINATION features
    out = (segment_sum(msg, dst) + bias.reshape(1,H,F)).mean(axis=1)

Key algebraic identity: because the message uses ft[dst] (not ft[src]),
every edge in dst-segment n contributes ft[n] * a_e, and the softmax
weights a_e of one segment sum to 1.  Hence

    segment_sum(msg, dst)[n] = ft[n] * (1 if node n has >=1 in-edge else 0)

exactly (up to f32 rounding of order 1e-7 -- verified global rel err
1.2e-7 vs the jax reference).  The attention logits, the e_ft @ W matmul
and the edge gathers cancel out of the output entirely; the only thing
the edge list contributes is the per-node "has in-edge" indicator.

So the kernel computes, fully on device:

    out[n, f] = (sum_h ft[n, h, f]) * fscale[n] + bias_mean[f]

where fscale[n] = 0.25 * has_in_edge[n] (the 1/H fold is free) and
bias_mean = bias.reshape(H, F).mean(0).  The indicator is produced on
the host during input sharding (a single vectorized scatter over dst --
index preprocessing, like the sharding itself).

Distribution: node-parallel across the 8 NeuronCores.  Each core gets a
12500-node shard (padded to 12544 = 98*128) and streams its 6.4 MB of
ft through SBUF, which makes the kernel purely HBM-bandwidth-bound --
the target regime.

Implementation is raw Bass (no Tile framework, no Block) with manual
semaphores; the Tile scheduler's entry/exit drain + all-engine barriers
cost ~15 us on a ~25 us kernel.  Pipeline (NBUF-deep rotating buffers):
  - SP (sync) HWDGE ring:    7x 896 KB ft tile loads, free-running
  - ACT (scalar) HWDGE ring: fscale/bias const loads + 7x 229 KB stores
    (separate ring so stores' sem-waits never block load issue)
  - DVE (vector) per tile:   u=h0+h2, v=h1+h3, o=u+v, o*=fscale_bcast
  - GpSimd per tile:         o+=bias_bcast (parallel with DVE's next
    tile); plus the end-of-kernel semaphore clear (gated on per-engine
    done incs) so the loaded NEFF stays re-executable.
The Bass-constructor entry all-engine-barrier (it only protects four
const-tile memsets this kernel never reads) is patched out during
construction -- all cross-engine ordering here is explicit via the
kernel's own semaphores.

Semaphore-counting note: one HWDGE DMA fans its 16 per-SDMA-engine
slices' completions into the sem as +1 each; "sem >= 16*m" therefore
proves ALL first m DMAs on that ring are complete (every engine must
have retired its slice of every one of them), but "sem >= 16" with two
DMAs in flight does NOT prove the first one finished.
"""

import numpy as np

N = 100000
H = 4
F = 32
D = H * F            # 128 floats per node in ft
NC = 8               # cores
PER = N // NC        # 12500 nodes per core
P = 128              # SBUF partitions
G = 14               # node-groups per partition per tile
B = 7                # tiles per core
PAD = P * G * B      # 12544 padded nodes per core
NBUF = 3             # ft / out buffer slots

_cached = None


def _make_nc():
    """Construct the Bass object with the init-time all-engine barrier and
    const-tile memsets suppressed (nothing in this kernel reads them; all
    cross-engine ordering is via the kernel's own semaphores)."""
    import concourse.bass as bass

    orig_aeb = bass.Bass.all_engine_barrier
    orig_memset = bass.BassSharedVectorInterface.memset
    bass.Bass.all_engine_barrier = lambda self, **kw: None
    bass.BassSharedVectorInterface.memset = lambda self, ap, c: None
    try:
        nc = bass.Bass(
            "TRN2",
            target_bir_lowering=False,
            debug=False,
            num_devices=NC,
        )
    finally:
        bass.Bass.all_engine_barrier = orig_aeb
        bass.BassSharedVectorInterface.memset = orig_memset
    return nc


def _build_bass():
    import concourse.bass as bass
    from concourse import mybir

    f32 = mybir.dt.float32
    nc = _make_nc()
    ft_in = nc.dram_tensor("ft_in", [PAD, D], f32, kind="ExternalInput").ap()
    fs_in = nc.dram_tensor("fs_in", [PAD], f32, kind="ExternalInput").ap()
    bias_in = nc.dram_tensor("bias_in", [P, F], f32, kind="ExternalInput").ap()
    out = nc.dram_tensor("out", [PAD, F], f32, kind="ExternalOutput").ap()

    # node index n (within the core's shard) = p*(G*B) + b*G + g
    ftv = ft_in.rearrange("(p b g) d -> b p (g d)", p=P, b=B, g=G)   # [B,128,G*D]
    fsv = fs_in.rearrange("(p x) -> p x", p=P)                        # [128, B*G]
    outv = out.rearrange("(p b g) f -> b p (g f)", p=P, b=B, g=G)     # [B,128,G*F]

    sem_ft = nc.alloc_semaphore("sem_ft")        # ft loads      (+16, SP ring)
    sem_cb = nc.alloc_semaphore("sem_cb")        # consts+stores (+16, ACT ring)
    sem_ftfree = nc.alloc_semaphore("sem_ftfree")  # vector done reading ft (+1)
    sem_v4 = nc.alloc_semaphore("sem_v4")        # vector finished tile (+1)
    sem_comp = nc.alloc_semaphore("sem_comp")    # gpsimd bias-add done (+1)
    sem_done = nc.alloc_semaphore("sem_done")    # per-engine finished (+1)
    ALL_SEMS = [sem_ft, sem_cb, sem_ftfree, sem_v4, sem_comp, sem_done]

    GD, GF = G * D, G * F

    with (
        nc.sbuf_tensor("ft_buf", [P, NBUF * GD], f32) as ft_buf,
        nc.sbuf_tensor("u_buf", [P, 2 * GF], f32) as u_buf,
        nc.sbuf_tensor("o_buf", [P, NBUF * GF], f32) as o_buf,
        nc.sbuf_tensor("fs_buf", [P, B * G], f32) as fs_buf,
        nc.sbuf_tensor("bias_buf", [P, F], f32) as bias_buf,
    ):
        def o3(b):
            return (
                o_buf[:, (b % NBUF) * GF : (b % NBUF + 1) * GF]
                .rearrange("p (g f) -> p g f", f=F)
            )

        # ---- SP ring: ft tile loads ------------------------------------
        for b in range(B):
            ld = nc.sync.dma_start(
                ft_buf[:, (b % NBUF) * GD : (b % NBUF + 1) * GD], ftv[b]
            )
            if b >= NBUF:
                ld._wait_ge(sem_ftfree, b - NBUF + 1)
            ld.then_inc(sem_ft, 16)
        nc.sync.wait_ge(sem_ft, 16 * B).then_inc(sem_done, 1)

        # ---- ACT ring: consts then stores ------------------------------
        nc.scalar.dma_start(fs_buf[:], fsv).then_inc(sem_cb, 16)
        nc.scalar.dma_start(bias_buf[:], bias_in).then_inc(sem_cb, 16)
        for b in range(B):
            st = nc.scalar.dma_start(
                outv[b], o_buf[:, (b % NBUF) * GF : (b % NBUF + 1) * GF]
            )
            st._wait_ge(sem_comp, b + 1)
            st.then_inc(sem_cb, 16)
        nc.scalar.wait_ge(sem_cb, 16 * (B + 2)).then_inc(sem_done, 1)

        # ---- DVE: head sums + fscale multiply --------------------------
        for b in range(B):
            ft_t = ft_buf[:, (b % NBUF) * GD : (b % NBUF + 1) * GD]
            fth = ft_t.rearrange("p (g hh f) -> p hh g f", g=G, hh=H)
            u2 = u_buf[:, :GF].rearrange("p (g f) -> p g f", f=F)
            v2 = u_buf[:, GF:].rearrange("p (g f) -> p g f", f=F)
            op1 = nc.vector.tensor_add(u2, fth[:, 0], fth[:, 2])
            op1._wait_ge(sem_ft, 16 * (b + 1))
            op2 = nc.vector.tensor_add(v2, fth[:, 1], fth[:, 3])
            op2.then_inc(sem_ftfree, 1)
            op3 = nc.vector.tensor_add(o3(b), u2, v2)
            if b >= NBUF:
                # o slot free once store of tile b-NBUF retired
                # (store b-NBUF is DMA #(2 + b - NBUF + 1) on the ACT ring)
                op3._wait_ge(sem_cb, 16 * (2 + b - NBUF + 1))
            fs_bc = (
                fs_buf[:, b * G : (b + 1) * G].unsqueeze(2).broadcast_to([P, G, F])
            )
            op4 = nc.vector.tensor_mul(o3(b), o3(b), fs_bc)
            if b == 0:
                op4._wait_ge(sem_cb, 32)  # both const DMAs fully retired
            op4.then_inc(sem_v4, 1)
        nc.vector.wait_ge(sem_v4, B).then_inc(sem_done, 1)

        # ---- GpSimd: bias adds + final sem clear -----------------------
        bias_bc = bias_buf[:].unsqueeze(1).broadcast_to([P, G, F])
        for b in range(B):
            ba = nc.gpsimd.tensor_add(o3(b), o3(b), bias_bc)
            ba._wait_ge(sem_v4, b + 1)
            ba.then_inc(sem_comp, 1)
        # end-of-kernel: wait for every engine, then zero all semaphores so
        # the loaded NEFF can be executed again.
        nc.gpsimd.sem_clear(ALL_SEMS[0])._wait_ge(sem_done, 3)
        for s in ALL_SEMS[1:]:
            nc.gpsimd.sem_clear(s)

    return nc


# results of the last device run (for test harness introspection)
LAST_RESULTS = None


def kernel(ft, e_ft, W, bias, src, dst):
    global _cached, LAST_RESULTS
    from concourse import bass_utils

    ft = np.ascontiguousarray(np.asarray(ft, dtype=np.float32)).reshape(N, D)
    bias = np.asarray(bias, dtype=np.float32)
    dst = np.asarray(dst)

    # per-node in-edge indicator, folded with the 1/H of the head mean
    fscale = np.zeros(N, np.float32)
    fscale[dst] = 1.0 / H
    bias_mean = bias.reshape(H, F).mean(axis=0)
    bias_b = np.ascontiguousarray(np.broadcast_to(bias_mean, (P, F)))

    in_maps = []
    for c in range(NC):
        ft_s = np.zeros((PAD, D), np.float32)
        ft_s[:PER] = ft[c * PER : (c + 1) * PER]
        fs_s = np.zeros(PAD, np.float32)
        fs_s[:PER] = fscale[c * PER : (c + 1) * PER]
        in_maps.append({"ft_in": ft_s, "fs_in": fs_s, "bias_in": bias_b})

    if _cached is None:
        _cached = _build_bass()
    nc = _cached

    res = bass_utils.run_bass_kernel_spmd(nc, in_maps, core_ids=list(range(NC)))
    LAST_RESULTS = res
    out = np.empty((N, F), np.float32)
    for c in range(NC):
        out[c * PER : (c + 1) * PER] = res.results[c]["out"][:PER]
    return out


# revision 9
# speedup vs baseline: 1.4167x; 1.0946x over previous
"""Trainium2 Bass kernel for nn_CDER_64493228917301 (gnn_message_passing).

Reference semantics (GATConv-style, DGL u_dot_v / v_mul_e):
    el  = (e_ft @ W.T).reshape(N, H, F)
    e   = leaky_relu(einsum('ehf,ehf->eh', el[src], el[dst]))
    a   = segment_softmax(e, dst)          # softmax over edges sharing dst
    msg = ft[dst] * a[:, :, None]          # NOTE: uses DESTINATION features
    out = (segment_sum(msg, dst) + bias.reshape(1,H,F)).mean(axis=1)

Key algebraic identity: because the message uses ft[dst] (not ft[src]),
every edge in dst-segment n contributes ft[n] * a_e, and the softmax
weights a_e of one segment sum to 1.  Hence

    segment_sum(msg, dst)[n] = ft[n] * (1 if node n has >=1 in-edge else 0)

exactly (up to f32 rounding of order 1e-7 -- verified global rel err
1.2e-7 vs the jax reference).  The attention logits, the e_ft @ W matmul
and the edge gathers cancel out of the output entirely; the only thing
the edge list contributes is the per-node "has in-edge" indicator.

So the kernel computes, fully on device:

    out[n, f] = (sum_h ft[n, h, f]) * fscale[n] + bias_mean[f]

where fscale[n] = 0.25 * has_in_edge[n] (the 1/H fold is free) and
bias_mean = bias.reshape(H, F).mean(0).  The indicator is produced on
the host during input sharding (a single vectorized scatter over dst --
index preprocessing, like the sharding itself).

Distribution: node-parallel across the 8 NeuronCores.  Each core gets a
12500-node shard (padded to 12544 = 98*128) and streams its 6.4 MB of
ft through SBUF, which makes the kernel purely HBM-bandwidth-bound --
the target regime.

Implementation is raw Bass (no Tile framework, no Block) with manual
semaphores; the Tile scheduler's entry/exit drain + all-engine barriers
cost ~15 us on a ~25 us kernel.  Pipeline (rotating SBUF slots, tiles
sized [7,14,14,14,14,14,14,7] node-groups so the pipeline ramp and the
post-last-load serial chain are both ~half a regular tile):
  - SP (sync) HWDGE ring:    8 ft tile loads, free-running
  - ACT (scalar) HWDGE ring: fscale load + 8 stores (separate ring so
    stores' sem-waits never block load issue)
  - DVE (vector) per tile:   u=h0+h2, v=h1+h3, o=u+v, o*=fscale_bcast
  - GpSimd:                  end-of-kernel semaphore clear (gated on
    per-engine done incs) so the loaded NEFF stays re-executable.
When bias is nonzero (never for this generator, which fills it with
zeros), a separate prebuilt variant adds a GpSimd bias-add stage
between the DVE multiply and the store.

The Bass-constructor entry all-engine-barrier (it only protects const
tiles this kernel never reads) is patched out during construction --
all cross-engine ordering here is explicit via the kernel's own
semaphores.

DMA completion counting: a DMA's 16 per-SDMA-engine slices each +1 the
semaphore, and engines drain their queues FIFO but with arbitrary
relative skew.  A cumulative threshold like "ring sem >= 16*m" is NOT
sound once later DMAs are in flight on the same sem: one engine can sit
mid-DMA-m while the other 15 race ahead and supply the missing incs
from DMA m+1.  (This bit as a one-node-per-run flaky corruption at the
compute/load convergence point.)  Sound scheme used here: one semaphore
per rotating buffer slot, so at most ONE DMA is ever in flight per
semaphore and "slot sem >= 16*k" exactly means the k-th DMA on that
slot retired.  DMA access patterns are kept strictly 2D
[partition, contiguous-free] so every transfer engages all 16 SDMA
engines uniformly.
"""

import numpy as np

N = 100000
H = 4
F = 32
D = H * F            # 128 floats per node in ft
NC = 8               # cores
PER = N // NC        # 12500 nodes per core
P = 128              # SBUF partitions
X = 98               # nodes per partition
PAD = P * X          # 12544 padded nodes per core
GS = [7, 14, 14, 14, 14, 14, 14, 7]          # tile sizes in node-groups
XS = [0, 7, 21, 35, 49, 63, 77, 91]          # tile offsets
BT = len(GS)
GMAX = max(GS)
NBUF = 3             # ft / out buffer slots

_cached = {}


def _make_nc():
    """Construct the Bass object with the init-time all-engine barrier
    suppressed (it only guards const-tile memsets this kernel never reads;
    all cross-engine ordering is via the kernel's own semaphores)."""
    import concourse.bass as bass

    orig_aeb = bass.Bass.all_engine_barrier
    bass.Bass.all_engine_barrier = lambda self, **kw: None
    try:
        nc = bass.Bass(
            "TRN2",
            target_bir_lowering=False,
            debug=False,
            num_devices=NC,
        )
    finally:
        bass.Bass.all_engine_barrier = orig_aeb
    return nc


def _build_bass(with_bias: bool):
    from concourse import mybir

    f32 = mybir.dt.float32
    nc = _make_nc()
    ft_in = nc.dram_tensor("ft_in", [PAD, D], f32, kind="ExternalInput").ap()
    fs_in = nc.dram_tensor("fs_in", [PAD], f32, kind="ExternalInput").ap()
    bias_in = nc.dram_tensor("bias_in", [P, F], f32, kind="ExternalInput").ap()
    out = nc.dram_tensor("out", [PAD, F], f32, kind="ExternalOutput").ap()

    # node index n (within the core's shard) = p*X + x
    ftd = ft_in.rearrange("(p x) d -> p (x d)", p=P)  # [128, 98*128]
    fsv = fs_in.rearrange("(p x) -> p x", p=P)        # [128, 98]
    outd = out.rearrange("(p x) f -> p (x f)", p=P)   # [128, 98*32]

    # per-slot DMA-completion sems (at most one DMA in flight per sem)
    sem_fts = [nc.alloc_semaphore(f"sem_fts{s}") for s in range(NBUF)]
    sem_ost = [nc.alloc_semaphore(f"sem_ost{s}") for s in range(NBUF)]
    sem_fs = nc.alloc_semaphore("sem_fs")        # fscale const load
    sem_bs = nc.alloc_semaphore("sem_bs")        # bias const load
    sem_ftfree = nc.alloc_semaphore("sem_ftfree")  # vector done reading ft (+1)
    sem_v4 = nc.alloc_semaphore("sem_v4")        # vector finished tile (+1)
    sem_comp = nc.alloc_semaphore("sem_comp")    # gpsimd bias-add done (+1)
    sem_done = nc.alloc_semaphore("sem_done")    # per-engine finished (+1)
    ALL_SEMS = sem_fts + sem_ost + [
        sem_fs, sem_bs, sem_ftfree, sem_v4, sem_comp, sem_done
    ]

    # which (+1)-sem gates a store: gpsimd bias-add done vs vector done
    sem_store_gate = sem_comp if with_bias else sem_v4

    def nslot(b):
        """how many tile-indices <= b map to slot b%NBUF"""
        return b // NBUF + 1

    with (
        nc.sbuf_tensor("ft_buf", [P, NBUF * GMAX * D], f32) as ft_buf,
        nc.sbuf_tensor("u_buf", [P, 2 * GMAX * F], f32) as u_buf,
        nc.sbuf_tensor("o_buf", [P, NBUF * GMAX * F], f32) as o_buf,
        nc.sbuf_tensor("fs_buf", [P, X], f32) as fs_buf,
        nc.sbuf_tensor("bias_buf", [P, F], f32) as bias_buf,
    ):
        def ft_t(b):
            s = (b % NBUF) * GMAX * D
            return ft_buf[:, s : s + GS[b] * D]

        def o2(b):
            s = (b % NBUF) * GMAX * F
            return o_buf[:, s : s + GS[b] * F]

        def o3(b):
            return o2(b).rearrange("p (g f) -> p g f", f=F)

        # ---- SP ring: ft tile loads ------------------------------------
        for b in range(BT):
            src = ftd[:, XS[b] * D : (XS[b] + GS[b]) * D]
            ld = nc.sync.dma_start(ft_t(b), src)
            if b >= NBUF:
                ld._wait_ge(sem_ftfree, b - NBUF + 1)
            ld.then_inc(sem_fts[b % NBUF], 16)
        fin = nc.sync.wait_ge(sem_fts[0], 16 * sum(1 for b in range(BT) if b % NBUF == 0))
        for s in range(1, NBUF):
            fin = nc.sync.wait_ge(
                sem_fts[s], 16 * sum(1 for b in range(BT) if b % NBUF == s)
            )
        fin.then_inc(sem_done, 1)

        # ---- ACT ring: const(s) then stores ----------------------------
        nc.scalar.dma_start(fs_buf[:], fsv).then_inc(sem_fs, 16)
        if with_bias:
            nc.scalar.dma_start(bias_buf[:], bias_in).then_inc(sem_bs, 16)
        for b in range(BT):
            st = nc.scalar.dma_start(
                outd[:, XS[b] * F : (XS[b] + GS[b]) * F], o2(b)
            )
            st._wait_ge(sem_store_gate, b + 1)
            st.then_inc(sem_ost[b % NBUF], 16)
        fin = nc.scalar.wait_ge(
            sem_ost[0], 16 * sum(1 for b in range(BT) if b % NBUF == 0)
        )
        for s in range(1, NBUF):
            fin = nc.scalar.wait_ge(
                sem_ost[s], 16 * sum(1 for b in range(BT) if b % NBUF == s)
            )
        fin.then_inc(sem_done, 1)

        # ---- DVE: head sums + fscale multiply --------------------------
        for b in range(BT):
            g = GS[b]
            fth = ft_t(b).rearrange("p (g hh f) -> p hh g f", g=g, hh=H)
            u2 = u_buf[:, : g * F].rearrange("p (g f) -> p g f", f=F)
            v2 = u_buf[:, GMAX * F : (GMAX + g) * F].rearrange(
                "p (g f) -> p g f", f=F
            )
            op1 = nc.vector.tensor_add(u2, fth[:, 0], fth[:, 2])
            op1._wait_ge(sem_fts[b % NBUF], 16 * nslot(b))
            op2 = nc.vector.tensor_add(v2, fth[:, 1], fth[:, 3])
            op2.then_inc(sem_ftfree, 1)
            op3 = nc.vector.tensor_add(o3(b), u2, v2)
            if b >= NBUF:
                # o slot free once the previous store from this slot retired
                op3._wait_ge(sem_ost[b % NBUF], 16 * (b // NBUF))
            fs_bc = (
                fs_buf[:, XS[b] : XS[b] + g].unsqueeze(2).broadcast_to([P, g, F])
            )
            op4 = nc.vector.tensor_mul(o3(b), o3(b), fs_bc)
            if b == 0:
                op4._wait_ge(sem_fs, 16)
            op4.then_inc(sem_v4, 1)
        nc.vector.wait_ge(sem_v4, BT).then_inc(sem_done, 1)

        # ---- GpSimd: (optional bias adds) + final sem clear ------------
        if with_bias:
            bias_bc = bias_buf[:].unsqueeze(1).broadcast_to([P, GMAX, F])
            for b in range(BT):
                g = GS[b]
                ba = nc.gpsimd.tensor_add(o3(b), o3(b), bias_bc[:, :g, :])
                ba._wait_ge(sem_bs if b == 0 else sem_v4, 16 if b == 0 else b + 1)
                if b == 0:
                    # separate wait for vector (the attached slot was used
                    # by the bias-load wait)
                    ba.wait_op(sem_v4, 1, "sem-ge")
                ba.then_inc(sem_comp, 1)
        # end-of-kernel: wait for every engine, then zero all semaphores so
        # the loaded NEFF can be executed again.
        nc.gpsimd.sem_clear(ALL_SEMS[0])._wait_ge(sem_done, 3)
        for s in ALL_SEMS[1:]:
            nc.gpsimd.sem_clear(s)

    return nc


# results of the last device run (for test harness introspection)
LAST_RESULTS = None


def kernel(ft, e_ft, W, bias, src, dst):
    global LAST_RESULTS
    from concourse import bass_utils

    ft = np.ascontiguousarray(np.asarray(ft, dtype=np.float32)).reshape(N, D)
    bias = np.asarray(bias, dtype=np.float32)
    dst = np.asarray(dst)

    # per-node in-edge indicator, folded with the 1/H of the head mean
    fscale = np.zeros(N, np.float32)
    fscale[dst] = 1.0 / H
    with_bias = bool(np.any(bias))
    bias_mean = bias.reshape(H, F).mean(axis=0)
    bias_b = np.ascontiguousarray(np.broadcast_to(bias_mean, (P, F)))

    in_maps = []
    for c in range(NC):
        ft_s = np.zeros((PAD, D), np.float32)
        ft_s[:PER] = ft[c * PER : (c + 1) * PER]
        fs_s = np.zeros(PAD, np.float32)
        fs_s[:PER] = fscale[c * PER : (c + 1) * PER]
        in_maps.append({"ft_in": ft_s, "fs_in": fs_s, "bias_in": bias_b})

    if with_bias not in _cached:
        _cached[with_bias] = _build_bass(with_bias)
    nc = _cached[with_bias]

    res = bass_utils.run_bass_kernel_spmd(nc, in_maps, core_ids=list(range(NC)))
    LAST_RESULTS = res
    out = np.empty((N, F), np.float32)
    for c in range(NC):
        out[c * PER : (c + 1) * PER] = res.results[c]["out"][:PER]
    return out


# revision 10
# speedup vs baseline: 1.4327x; 1.0113x over previous
"""Trainium2 Bass kernel for nn_CDER_64493228917301 (gnn_message_passing).

Reference semantics (GATConv-style, DGL u_dot_v / v_mul_e):
    el  = (e_ft @ W.T).reshape(N, H, F)
    e   = leaky_relu(einsum('ehf,ehf->eh', el[src], el[dst]))
    a   = segment_softmax(e, dst)          # softmax over edges sharing dst
    msg = ft[dst] * a[:, :, None]          # NOTE: uses DESTINATION features
    out = (segment_sum(msg, dst) + bias.reshape(1,H,F)).mean(axis=1)

Key algebraic identity: because the message uses ft[dst] (not ft[src]),
every edge in dst-segment n contributes ft[n] * a_e, and the softmax
weights a_e of one segment sum to 1.  Hence

    segment_sum(msg, dst)[n] = ft[n] * (1 if node n has >=1 in-edge else 0)

exactly (up to f32 rounding of order 1e-7 -- verified global rel err
1.2e-7 vs the jax reference).  The attention logits, the e_ft @ W matmul
and the edge gathers cancel out of the output entirely; the only thing
the edge list contributes is the per-node "has in-edge" indicator.

So the kernel computes, fully on device:

    out[n, f] = (sum_h ft[n, h, f]) * fscale[n] + bias_mean[f]

where fscale[n] = 0.25 * has_in_edge[n] (the 1/H fold is free) and
bias_mean = bias.reshape(H, F).mean(0).  The indicator is produced on
the host during input sharding (a single vectorized scatter over dst --
index preprocessing, like the sharding itself).

Distribution: node-parallel across the 8 NeuronCores.  Each core gets a
12500-node shard (padded to 12544 = 98*128) and streams its 6.4 MB of
ft through SBUF, which makes the kernel purely HBM-bandwidth-bound --
the target regime.

Implementation is raw Bass (no Tile framework, no Block) with manual
semaphores; the Tile scheduler's entry/exit drain + all-engine barriers
cost ~15 us on a ~25 us kernel.  Pipeline (rotating SBUF slots, tiles
sized [7,14,14,14,14,14,14,7] node-groups so the pipeline ramp and the
post-last-load serial chain are both ~half a regular tile):
  - SP (sync) HWDGE ring:    8 ft tile loads, free-running
  - ACT (scalar) HWDGE ring: fscale load + 8 stores (separate ring so
    stores' sem-waits never block load issue)
  - DVE (vector) per tile:   u=h0+h2, v=h1+h3, o=u+v, o*=fscale_bcast
  - GpSimd:                  end-of-kernel semaphore clear (gated on
    per-engine done incs) so the loaded NEFF stays re-executable.
When bias is nonzero (never for this generator, which fills it with
zeros), a separate prebuilt variant adds a GpSimd bias-add stage
between the DVE multiply and the store.

The Bass-constructor entry all-engine-barrier (it only protects const
tiles this kernel never reads) is patched out during construction --
all cross-engine ordering here is explicit via the kernel's own
semaphores.

DMA completion counting: a DMA's 16 per-SDMA-engine slices each +1 the
semaphore, and engines drain their queues FIFO but with arbitrary
relative skew.  A cumulative threshold like "ring sem >= 16*m" is NOT
sound once later DMAs are in flight on the same sem: one engine can sit
mid-DMA-m while the other 15 race ahead and supply the missing incs
from DMA m+1.  (This bit as a one-node-per-run flaky corruption at the
compute/load convergence point.)  Sound scheme used here: one semaphore
per rotating buffer slot, so at most ONE DMA is ever in flight per
semaphore and "slot sem >= 16*k" exactly means the k-th DMA on that
slot retired.  DMA access patterns are kept strictly 2D
[partition, contiguous-free] so every transfer engages all 16 SDMA
engines uniformly.
"""

import numpy as np

N = 100000
H = 4
F = 32
D = H * F            # 128 floats per node in ft
NC = 8               # cores
PER = N // NC        # 12500 nodes per core
P = 128              # SBUF partitions
X = 98               # nodes per partition
PAD = P * X          # 12544 padded nodes per core
GS = [7, 14, 14, 14, 14, 14, 14, 7]          # tile sizes in node-groups
XS = [0, 7, 21, 35, 49, 63, 77, 91]          # tile offsets
BT = len(GS)
GMAX = max(GS)
NBUF = 3             # ft / out buffer slots

_cached = {}


def _make_nc():
    """Construct the Bass object with the init-time all-engine barrier
    suppressed (it only guards const-tile memsets this kernel never reads;
    all cross-engine ordering is via the kernel's own semaphores)."""
    import concourse.bass as bass

    orig_aeb = bass.Bass.all_engine_barrier
    bass.Bass.all_engine_barrier = lambda self, **kw: None
    try:
        nc = bass.Bass(
            "TRN2",
            target_bir_lowering=False,
            debug=False,
            enable_asserts=False,
            num_devices=NC,
        )
    finally:
        bass.Bass.all_engine_barrier = orig_aeb
    return nc


def _build_bass(with_bias: bool):
    from concourse import mybir

    f32 = mybir.dt.float32
    nc = _make_nc()
    ft_in = nc.dram_tensor("ft_in", [PAD, D], f32, kind="ExternalInput").ap()
    fs_in = nc.dram_tensor("fs_in", [PAD], f32, kind="ExternalInput").ap()
    bias_in = nc.dram_tensor("bias_in", [P, F], f32, kind="ExternalInput").ap()
    out = nc.dram_tensor("out", [PAD, F], f32, kind="ExternalOutput").ap()

    # node index n (within the core's shard) = p*X + x
    ftd = ft_in.rearrange("(p x) d -> p (x d)", p=P)  # [128, 98*128]
    fsv = fs_in.rearrange("(p x) -> p x", p=P)        # [128, 98]
    outd = out.rearrange("(p x) f -> p (x f)", p=P)   # [128, 98*32]

    # per-slot DMA-completion sems (at most one DMA in flight per sem)
    sem_fts = [nc.alloc_semaphore(f"sem_fts{s}") for s in range(NBUF)]
    sem_ost = [nc.alloc_semaphore(f"sem_ost{s}") for s in range(NBUF)]
    sem_fs = nc.alloc_semaphore("sem_fs")        # fscale const load
    sem_bs = nc.alloc_semaphore("sem_bs")        # bias const load
    sem_ftfree = nc.alloc_semaphore("sem_ftfree")  # vector done reading ft (+1)
    sem_v4 = nc.alloc_semaphore("sem_v4")        # vector finished tile (+1)
    sem_comp = nc.alloc_semaphore("sem_comp")    # gpsimd bias-add done (+1)
    sem_done = nc.alloc_semaphore("sem_done")    # per-engine finished (+1)
    ALL_SEMS = sem_fts + sem_ost + [
        sem_fs, sem_bs, sem_ftfree, sem_v4, sem_comp, sem_done
    ]

    # which (+1)-sem gates a store: gpsimd bias-add done vs vector done
    sem_store_gate = sem_comp if with_bias else sem_v4

    def nslot(b):
        """how many tile-indices <= b map to slot b%NBUF"""
        return b // NBUF + 1

    with (
        nc.sbuf_tensor("ft_buf", [P, NBUF * GMAX * D], f32) as ft_buf,
        nc.sbuf_tensor("u_buf", [P, 2 * GMAX * F], f32) as u_buf,
        nc.sbuf_tensor("o_buf", [P, NBUF * GMAX * F], f32) as o_buf,
        nc.sbuf_tensor("fs_buf", [P, X], f32) as fs_buf,
        nc.sbuf_tensor("bias_buf", [P, F], f32) as bias_buf,
    ):
        def ft_t(b):
            s = (b % NBUF) * GMAX * D
            return ft_buf[:, s : s + GS[b] * D]

        def o2(b):
            s = (b % NBUF) * GMAX * F
            return o_buf[:, s : s + GS[b] * F]

        def o3(b):
            return o2(b).rearrange("p (g f) -> p g f", f=F)

        # ---- SP ring: ft tile loads ------------------------------------
        for b in range(BT):
            src = ftd[:, XS[b] * D : (XS[b] + GS[b]) * D]
            ld = nc.sync.dma_start(ft_t(b), src)
            if b >= NBUF:
                ld._wait_ge(sem_ftfree, b - NBUF + 1)
            ld.then_inc(sem_fts[b % NBUF], 16)
        fin = nc.sync.wait_ge(sem_fts[0], 16 * sum(1 for b in range(BT) if b % NBUF == 0))
        for s in range(1, NBUF):
            fin = nc.sync.wait_ge(
                sem_fts[s], 16 * sum(1 for b in range(BT) if b % NBUF == s)
            )
        fin.then_inc(sem_done, 1)

        # ---- ACT ring: const(s) then stores ----------------------------
        nc.scalar.dma_start(fs_buf[:], fsv).then_inc(sem_fs, 16)
        if with_bias:
            nc.scalar.dma_start(bias_buf[:], bias_in).then_inc(sem_bs, 16)
        for b in range(BT):
            st = nc.scalar.dma_start(
                outd[:, XS[b] * F : (XS[b] + GS[b]) * F], o2(b)
            )
            st._wait_ge(sem_store_gate, b + 1)
            st.then_inc(sem_ost[b % NBUF], 16)
        fin = nc.scalar.wait_ge(
            sem_ost[0], 16 * sum(1 for b in range(BT) if b % NBUF == 0)
        )
        for s in range(1, NBUF):
            fin = nc.scalar.wait_ge(
                sem_ost[s], 16 * sum(1 for b in range(BT) if b % NBUF == s)
            )
        fin.then_inc(sem_done, 1)

        # ---- DVE: head sums + fscale multiply --------------------------
        for b in range(BT):
            g = GS[b]
            fth = ft_t(b).rearrange("p (g hh f) -> p hh g f", g=g, hh=H)
            u2 = u_buf[:, : g * F].rearrange("p (g f) -> p g f", f=F)
            v2 = u_buf[:, GMAX * F : (GMAX + g) * F].rearrange(
                "p (g f) -> p g f", f=F
            )
            op1 = nc.vector.tensor_add(u2, fth[:, 0], fth[:, 2])
            op1._wait_ge(sem_fts[b % NBUF], 16 * nslot(b))
            op2 = nc.vector.tensor_add(v2, fth[:, 1], fth[:, 3])
            op2.then_inc(sem_ftfree, 1)
            op3 = nc.vector.tensor_add(o3(b), u2, v2)
            if b >= NBUF:
                # o slot free once the previous store from this slot retired
                op3._wait_ge(sem_ost[b % NBUF], 16 * (b // NBUF))
            fs_bc = (
                fs_buf[:, XS[b] : XS[b] + g].unsqueeze(2).broadcast_to([P, g, F])
            )
            op4 = nc.vector.tensor_mul(o3(b), o3(b), fs_bc)
            if b == 0:
                op4._wait_ge(sem_fs, 16)
            op4.then_inc(sem_v4, 1)
        nc.vector.wait_ge(sem_v4, BT).then_inc(sem_done, 1)

        # ---- GpSimd: (optional bias adds) + final sem clear ------------
        if with_bias:
            bias_bc = bias_buf[:].unsqueeze(1).broadcast_to([P, GMAX, F])
            for b in range(BT):
                g = GS[b]
                ba = nc.gpsimd.tensor_add(o3(b), o3(b), bias_bc[:, :g, :])
                ba._wait_ge(sem_bs if b == 0 else sem_v4, 16 if b == 0 else b + 1)
                if b == 0:
                    # separate wait for vector (the attached slot was used
                    # by the bias-load wait)
                    ba.wait_op(sem_v4, 1, "sem-ge")
                ba.then_inc(sem_comp, 1)
        # end-of-kernel: wait for every engine, then zero all semaphores so
        # the loaded NEFF can be executed again.
        nc.gpsimd.sem_clear(ALL_SEMS[0])._wait_ge(sem_done, 3)
        for s in ALL_SEMS[1:]:
            nc.gpsimd.sem_clear(s)

    return nc


# results of the last device run (for test harness introspection)
LAST_RESULTS = None


def kernel(ft, e_ft, W, bias, src, dst):
    global LAST_RESULTS
    from concourse import bass_utils

    ft = np.ascontiguousarray(np.asarray(ft, dtype=np.float32)).reshape(N, D)
    bias = np.asarray(bias, dtype=np.float32)
    dst = np.asarray(dst)

    # per-node in-edge indicator, folded with the 1/H of the head mean
    fscale = np.zeros(N, np.float32)
    fscale[dst] = 1.0 / H
    with_bias = bool(np.any(bias))
    bias_mean = bias.reshape(H, F).mean(axis=0)
    bias_b = np.ascontiguousarray(np.broadcast_to(bias_mean, (P, F)))

    in_maps = []
    for c in range(NC):
        ft_s = np.zeros((PAD, D), np.float32)
        ft_s[:PER] = ft[c * PER : (c + 1) * PER]
        fs_s = np.zeros(PAD, np.float32)
        fs_s[:PER] = fscale[c * PER : (c + 1) * PER]
        in_maps.append({"ft_in": ft_s, "fs_in": fs_s, "bias_in": bias_b})

    if with_bias not in _cached:
        _cached[with_bias] = _build_bass(with_bias)
    nc = _cached[with_bias]

    res = bass_utils.run_bass_kernel_spmd(nc, in_maps, core_ids=list(range(NC)))
    LAST_RESULTS = res
    out = np.empty((N, F), np.float32)
    for c in range(NC):
        out[c * PER : (c + 1) * PER] = res.results[c]["out"][:PER]
    return out
